# revision 7
# baseline (speedup 1.0000x reference)
"""DIFF cross-attention kernel for 8 Trainium2 NeuronCores.

Sharding: tensor-parallel over heads x data-parallel over batch.
Core r handles batch b = r//4 and head group g = r%4 (4 of 16 heads).

Device math (per core, channel-major "transposed" layout, bf16 matmul
operands everywhere, f32 PSUM accumulation):
  q1T/q2T [hd, Nq], k1T/k2T [hd, Nk]  via projections of query.T / key.T
  v1ext   [Nk, 65] per head (64 v-channels + ones column for softmax sums)
  scoresT [keys, q] = k1T_h.T @ q1T_h    (PE, K=64)
  eT = exp(scoresT * 1/8) -> bf16        (ACT, fused scale, no max-sub:
                                          scores are provably < ~27)
  u_ext [65, q] = v1ext.T @ eT           (PE, rows 0-63 = unnorm out,
                                          row 64 = softmax denominator)
  xT[ch, q] = u1/l1 - lambda*u2/l2       (DVE + gpsimd partition_broadcast)
  ss[q] = sum_ch x^2                     (DVE square + PE ones-matmul)

Pipeline: the attention loop is q-block-outer. After each q-block the
core's [xT | ss] rows (257x512, bf16) go out in an AllGather over the
4-core batch group; qb0's gather, projection and normalization all hide
under qb1's attention (post-AG consumers are emitted mid-way into qb1's
loop so the in-order PE queue never waits on the collective). RMSNorm's
column scale commutes through the projection (y = (W_nw x) * s + b), so
norm_w is folded into the proj weights on the host and the projection
runs on raw gathered x with K=1024, each core producing only its own
256 output rows -- no reduce. rsqrt is computed as exp(-0.5*ln(x)) so
the ACT engine stays on one function-table set (no table-switch stalls
in the exp stream).

Projection phases run as d-outer waves streaming behind the input DMAs;
the second hd-slice of the q/k projections is emitted interleaved into
the first heads' attention loop as PE filler work. Host returns
out[b][t, c] = y_out[r][c', t].
"""

import numpy as np
import ml_dtypes

B = 2
NQ = 1024
NK = 2048
DIM = 1024
H = 16
HD = 64
NH = 4            # heads per core
G = 4             # cores per batch group
SCALE = 0.125
LAMBDA_INIT = 0.1
EPS = 1e-6
P = 128
DC = DIM // P     # 8 contraction chunks
KT = NK // P      # 16 key tiles
GROUPS = [[0, 1, 2, 3], [4, 5, 6, 7]]
AGR = 2 * P + 1   # rows per rank in the AllGather payload
QH = NQ // 2      # q-block (half) size


def _build(stop_after="full", loop_n=0):
    import concourse.bass as bass
    import concourse.tile as tile
    import concourse.mybir as mybir
    from concourse import bacc

    f32 = mybir.dt.float32
    bf16 = mybir.dt.bfloat16
    AF = mybir.ActivationFunctionType

    nc = bacc.Bacc("TRN2", target_bir_lowering=False, debug=False, num_devices=8)

    qT_d = nc.dram_tensor("qT", [DIM, NQ], bf16, kind="ExternalInput")
    kT_d = nc.dram_tensor("kT", [DIM, NK], bf16, kind="ExternalInput")
    wq1_d = nc.dram_tensor("wq1", [DIM, 256], bf16, kind="ExternalInput")
    wq2_d = nc.dram_tensor("wq2", [DIM, 256], bf16, kind="ExternalInput")
    wk1_d = nc.dram_tensor("wk1", [DIM, 256], bf16, kind="ExternalInput")
    wv1_d = nc.dram_tensor("wv1", [DIM, 256], bf16, kind="ExternalInput")
    wk2_d = nc.dram_tensor("wk2", [DIM, 256], bf16, kind="ExternalInput")
    # proj weights with norm_w folded in, rows pre-permuted to the
    # AllGather chunk order: chunk j=(r, mi) -> channels 256r+128mi..+128,
    # cols = this core's 256 output channels.
    wp_d = nc.dram_tensor("wp", [DIM, 256], bf16, kind="ExternalInput")
    pb_d = nc.dram_tensor("pb", [P, 2], f32, kind="ExternalInput")
    lamn_d = nc.dram_tensor("lamn", [1, NH], f32, kind="ExternalInput")
    y_out_d = nc.dram_tensor("y_out", [256, NQ], f32, kind="ExternalOutput")

    def _trace(tc):
        with (
            tc.tile_pool(name="res", bufs=1) as res,
            tc.tile_pool(name="dram", bufs=1, space="DRAM") as dram,
        ):
            # ---- resident tensors; kT first so the projection waves can
            #      stream d-outer right behind the chunk DMAs ----
            kt_sb = res.tile([P, DC, NK], bf16)
            for d in range(DC):
                nc.sync.dma_start(kt_sb[:, d, :], kT_d[d * P:(d + 1) * P, :])
            qt_sb = res.tile([P, DC, NQ], bf16)
            for d in range(DC):
                nc.sync.dma_start(qt_sb[:, d, :], qT_d[d * P:(d + 1) * P, :])
            wq1 = res.tile([P, DC, 256], bf16, name="wq1s")
            wq2 = res.tile([P, DC, 256], bf16, name="wq2s")
            wk1 = res.tile([P, DC, 256], bf16, name="wk1s")
            wv1 = res.tile([P, DC, 256], bf16, name="wv1s")
            wk2 = res.tile([P, DC, 256], bf16, name="wk2s")
            wp_sb = res.tile([P, DC, 256], bf16, name="wps")
            for t_, d_ in ((wq1, wq1_d), (wq2, wq2_d), (wk1, wk1_d),
                           (wk2, wk2_d), (wv1, wv1_d), (wp_sb, wp_d)):
                for hh in range(2):
                    nc.sync.dma_start(
                        t_[:, hh * 4:(hh + 1) * 4, :],
                        d_[hh * 512:(hh + 1) * 512, :].rearrange(
                            "(o p) n -> p o n", p=P))
            pb = res.tile([P, 2], f32)
            nc.sync.dma_start(pb[:], pb_d[:])
            lamn = res.tile([1, NH], f32)
            nc.sync.dma_start(lamn[:], lamn_d[:])
            ones_f = res.tile([P, KT], bf16)
            nc.vector.memset(ones_f[:], 1.0)
            ones4 = res.tile([4, 1], bf16)
            nc.vector.memset(ones4[:], 1.0)
            eps_t = res.tile([1, 1], f32)
            nc.vector.memset(eps_t[:], EPS)

            q1T = res.tile([P, 2, NQ], bf16)
            q2T = res.tile([P, 2, NQ], bf16)
            k1T = res.tile([P, 2, NK], bf16)
            k2T = res.tile([P, 2, NK], bf16)
            v1e = res.tile([P, NH, KT, 65], bf16)
            for h in range(NH):
                nc.vector.tensor_copy(v1e[:, h, :, 64:65],
                                      ones_f[:].unsqueeze(-1))
            xT = res.tile([P, 2, NQ], bf16)

            ag_in = [dram.tile([AGR, QH], bf16, name=f"agi{qb}")
                     for qb in range(2)]
            ag_out = [dram.tile([G * AGR, QH], bf16, name=f"ago{qb}")
                      for qb in range(2)]

            with tc.tile_pool(name="ps_acc", bufs=8, space="PSUM") as ps_acc:
                # ---- projection waves, all d-outer so compute streams
                #      behind the per-chunk input DMAs ----
                def q_items(qc, pool):
                    for pj, wsrc, dst in ((0, wq1, q1T), (1, wq2, q2T)):
                        for m in range(2):
                            pst = pool.tile([P, 512], f32, tag="acc",
                                              name=f"q{qc}_{pj}{m}")
                            for d in range(DC):
                                yield lambda pst=pst, wsrc=wsrc, d=d, m=m: \
                                    nc.tensor.matmul(
                                        pst[:],
                                        wsrc[:, d, m * P:(m + 1) * P],
                                        qt_sb[:, d, qc * 512:(qc + 1) * 512],
                                        start=(d == 0), stop=(d == DC - 1),
                                    )
                            yield lambda pst=pst, dst=dst, m=m: \
                                nc.vector.tensor_copy(
                                    dst[:, m, qc * 512:(qc + 1) * 512],
                                    pst[:])

                def kproj_items(wsrcs, m, pool):
                    for wsrc, dst in wsrcs:
                        for kc in range(4):
                            pst = pool.tile([P, 512], f32, tag="acc",
                                            name=f"kp{m}_{kc}")
                            for d in range(DC):
                                yield lambda pst=pst, wsrc=wsrc, d=d, kc=kc, m=m: \
                                    nc.tensor.matmul(
                                        pst[:],
                                        wsrc[:, d, m * P:(m + 1) * P],
                                        kt_sb[:, d, kc * 512:(kc + 1) * 512],
                                        start=(d == 0), stop=(d == DC - 1),
                                    )
                            yield lambda pst=pst, dst=dst, kc=kc, m=m: \
                                nc.vector.tensor_copy(
                                    dst[:, m, kc * 512:(kc + 1) * 512], pst[:])

                def v_items(kts, pool):
                    for kt in kts:
                        pst = pool.tile([P, 512], f32, tag="acc",
                                        name=f"v{kt}")
                        for d in range(DC):
                            yield lambda pst=pst, d=d, kt=kt: \
                                nc.tensor.matmul(
                                    pst[:, 0:256],
                                    kt_sb[:, d, kt * P:(kt + 1) * P],
                                    wv1[:, d, :],
                                    start=(d == 0), stop=(d == DC - 1),
                                )
                        for h in range(NH):
                            yield lambda pst=pst, kt=kt, h=h: \
                                nc.vector.tensor_copy(
                                    v1e[:, h, kt, 0:64],
                                    pst[:, h * 64:h * 64 + 64])

                # W1: q projections qc=0 + k1 m=0 (8 accumulation banks)
                for it in q_items(0, ps_acc):
                    it()
                for it in kproj_items(((wk1, k1T),), 0, ps_acc):
                    it()
                # W2: k2 m=0 + v kt0-3
                for it in kproj_items(((wk2, k2T),), 0, ps_acc):
                    it()
                for it in v_items(range(0, 4), ps_acc):
                    it()
                # W3/W4: v kt4-15
                for it in v_items(range(4, 16), ps_acc):
                    it()

                if stop_after == "proj":
                    nc.sync.dma_start(y_out_d[0:P, 0:512],
                                      k1T[:, 0, :NQ].bitcast(f32))
                    return

            # ---- attention, qb-outer; filler = q qc=1 + k m=1 projections;
            #      per-qb tail = ss + AllGather + (proj + norm, emitted
            #      mid-way into the next qb's loop) ----
            with (
                tc.tile_pool(name="ps_sc", bufs=2, space="PSUM") as ps_sc,
                tc.tile_pool(name="ps_u", bufs=2, space="PSUM") as ps_u,
                tc.tile_pool(name="ps_fil", bufs=1, space="PSUM") as ps_fil,
                tc.tile_pool(name="ps_tl", bufs=1, space="PSUM") as ps_tl,
                tc.tile_pool(name="att", bufs=3) as att,
                tc.tile_pool(name="smal", bufs=2) as smal,
                tc.tile_pool(name="tl", bufs=2) as tl,
            ):
                import itertools
                filler = itertools.chain(
                    q_items(1, ps_fil),
                    kproj_items(((wk1, k1T), (wk2, k2T)), 1, ps_fil))
                fill_done = False

                def emit_fill(k):
                    nonlocal fill_done
                    if fill_done:
                        return
                    for _ in range(k):
                        it = next(filler, None)
                        if it is None:
                            fill_done = True
                            return
                        it()

                def attention(h, qb):
                    po = (h % 2) * 64
                    mi = h // 2
                    qs = slice(qb * 512, (qb + 1) * 512)
                    u_ps = []
                    for br, ktp, qtp in ((0, k1T, q1T), (1, k2T, q2T)):
                        u = ps_u.tile([65, 512], f32, tag="u")
                        u_ps.append(u)
                        for kg in range(KT // 2):
                            sc = ps_sc.tile([P, 1024], f32, tag="sc")
                            for j in range(2):
                                kt = kg * 2 + j
                                nc.tensor.matmul(
                                    sc[:, j * 512:(j + 1) * 512],
                                    ktp[po:po + 64, mi, kt * P:(kt + 1) * P],
                                    qtp[po:po + 64, mi, qs],
                                    start=True, stop=True,
                                )
                            e_t = att.tile([P, 1024], bf16, tag="e")
                            nc.scalar.activation(e_t[:], sc[:], AF.Exp,
                                                 scale=SCALE)
                            for j in range(2):
                                kt = kg * 2 + j
                                nc.tensor.matmul(
                                    u[:],
                                    v1e[:, h, kt, :],
                                    e_t[:, j * 512:(j + 1) * 512],
                                    start=(kt == 0), stop=(kt == KT - 1),
                                )
                            emit_fill(4)
                    # combine branches: x = u1/l1 - lambda*u2/l2
                    rr1 = smal.tile([1, 512], f32, tag="rr")
                    nc.vector.reciprocal(rr1[:], u_ps[0][64:65, :])
                    rr2 = smal.tile([1, 512], f32, tag="rr")
                    nc.vector.reciprocal(rr2[:], u_ps[1][64:65, :])
                    nc.vector.tensor_scalar_mul(rr2[:], rr2[:],
                                                lamn[0:1, h:h + 1])
                    rr1b = smal.tile([64, 512], f32, tag="rrb")
                    nc.gpsimd.partition_broadcast(rr1b[:], rr1[:])
                    rr2b = smal.tile([64, 512], f32, tag="rrb")
                    nc.gpsimd.partition_broadcast(rr2b[:], rr2[:])
                    t1 = smal.tile([64, 512], f32, tag="tt")
                    nc.vector.tensor_mul(t1[:], u_ps[0][0:64, :], rr1b[:])
                    t2 = smal.tile([64, 512], f32, tag="tt")
                    nc.vector.tensor_mul(t2[:], u_ps[1][0:64, :], rr2b[:])
                    nc.vector.tensor_add(xT[po:po + 64, mi, qs],
                                         t1[:], t2[:])

                def emit_tail1(qb):
                    """ss for this q-block, payload DMAs, AllGather, and
                    the gather read-back DMAs (these wait on the AG sem)."""
                    qs = slice(qb * 512, (qb + 1) * 512)
                    for mi in range(2):
                        nc.sync.dma_start(ag_in[qb][mi * P:(mi + 1) * P, :],
                                          xT[:, mi, qs])
                    ss_ps = ps_tl.tile([1, 512], f32, tag="tlp",
                                       name=f"ss{qb}")
                    for t in range(2):
                        x2c = tl.tile([P, 512], bf16, tag="x2")
                        nc.vector.tensor_mul(x2c[:], xT[:, t, qs],
                                             xT[:, t, qs])
                        nc.tensor.matmul(
                            ss_ps[:],
                            ones_f[:, 0:1],
                            x2c[:],
                            start=(t == 0), stop=(t == 1),
                        )
                    ss_sb = tl.tile([1, 512], bf16, tag="sssb")
                    nc.vector.tensor_copy(ss_sb[:], ss_ps[:])
                    nc.sync.dma_start(ag_in[qb][2 * P:2 * P + 1, :], ss_sb[:])
                    if stop_after in ("attn", "preag"):
                        return
                    nc.gpsimd.collective_compute(
                        "AllGather",
                        mybir.AluOpType.bypass,
                        replica_groups=GROUPS,
                        ins=[ag_in[qb].opt()],
                        outs=[ag_out[qb].opt()],
                    )
                    xall = tl.tile([P, DC, 512], bf16, tag="xall",
                                   name=f"xall{qb}")
                    for r in range(G):
                        for mi in range(2):
                            nc.sync.dma_start(
                                xall[:, r * 2 + mi, :],
                                ag_out[qb][r * AGR + mi * P:
                                           r * AGR + (mi + 1) * P, :])
                    ss4 = tl.tile([4, 512], bf16, tag="ss4", name=f"ss4_{qb}")
                    for r in range(G):
                        nc.sync.dma_start(
                            ss4[r:r + 1, :],
                            ag_out[qb][r * AGR + 2 * P:r * AGR + 2 * P + 1, :])
                    return xall, ss4

                def emit_tail2(qb, xall, ss4):
                    """Post-AllGather: norm scale s, projection, output."""
                    # s = rsqrt(mean ss + eps) = exp(-0.5 ln(...)); ln and
                    # exp share one ACT table set -> no table switch.
                    s_ps = ps_tl.tile([1, 512], f32, tag="tlp",
                                      name=f"s{qb}")
                    nc.tensor.matmul(s_ps[:], ones4[:], ss4[:],
                                     start=True, stop=True)
                    ln_row = tl.tile([1, 512], f32, tag="lnr")
                    nc.scalar.activation(ln_row[:], s_ps[:], AF.Ln,
                                         bias=eps_t[0:1, 0:1],
                                         scale=1.0 / DIM)
                    s_row = tl.tile([1, 512], f32, tag="srow")
                    nc.scalar.activation(s_row[:], ln_row[:], AF.Exp,
                                         scale=-0.5)
                    s_b = tl.tile([P, 512], f32, tag="sb")
                    nc.gpsimd.partition_broadcast(s_b[:], s_row[:])
                    for m in range(2):
                        yp = ps_tl.tile([P, 512], f32, tag="tlp",
                                        name=f"yp{qb}{m}")
                        for j in range(DC):
                            nc.tensor.matmul(
                                yp[:],
                                wp_sb[:, j, m * P:(m + 1) * P],
                                xall[:, j, :],
                                start=(j == 0), stop=(j == DC - 1),
                            )
                        y_sb = tl.tile([P, 512], f32, tag="ysb")
                        nc.vector.tensor_mul(y_sb[:], yp[:], s_b[:])
                        nc.vector.tensor_scalar_add(y_sb[:], y_sb[:],
                                                    pb[:, m:m + 1])
                        nc.sync.dma_start(
                            y_out_d[m * P:(m + 1) * P,
                                    qb * 512:(qb + 1) * 512],
                            y_sb[:])

                tails = {}
                for qb in range(2):
                    for h in range(NH):
                        attention(h, qb)
                        if qb == 1 and h == 2 and 0 in tails:
                            emit_tail2(0, *tails[0])
                    r = emit_tail1(qb)
                    if r is not None:
                        tails[qb] = r

                if stop_after in ("attn", "preag"):
                    nc.sync.dma_start(y_out_d[0:P, 0:512],
                                      xT[:, 0, :].bitcast(f32))
                    return
                if 1 in tails:
                    emit_tail2(1, *tails[1])

    with tile.TileContext(nc) as tc:
        if loop_n:
            with tc.For_i(0, loop_n, 1):
                _trace(tc)
        else:
            _trace(tc)
    nc.compile()
    return nc


_CACHE = {}


def _get_nc():
    if "nc" not in _CACHE:
        _CACHE["nc"] = _build()
    return _CACHE["nc"]


def _shard_inputs(inputs):
    bf = ml_dtypes.bfloat16
    q = np.asarray(inputs["query"], np.float32)
    k = np.asarray(inputs["key"], np.float32)
    q1_w = np.asarray(inputs["q1_w"], np.float32)
    q2_w = np.asarray(inputs["q2_w"], np.float32)
    kv1_w = np.asarray(inputs["kv1_w"], np.float32)
    kv2_w = np.asarray(inputs["kv2_w"], np.float32)
    proj_w = np.asarray(inputs["proj_w"], np.float32)
    proj_b = np.asarray(inputs["proj_b"], np.float32)
    norm_w = np.asarray(inputs["norm_w"], np.float32)
    lam1 = np.asarray(inputs["lambda_1"], np.float32).reshape(H)
    lam2 = np.asarray(inputs["lambda_2"], np.float32).reshape(H)
    lam_full = lam1 - lam2 + LAMBDA_INIT

    # proj with norm folded in: y[o, q] = sum_c wpnw[c, o] x[c, q]
    wpnw = (proj_w * norm_w[None, :]).T  # [c, o]

    def c(x):
        return np.ascontiguousarray(x).astype(bf)

    in_maps = []
    for r in range(8):
        b, g = r // G, r % G
        rows = slice(g * 256, (g + 1) * 256)
        vrows = slice(DIM + g * 256, DIM + (g + 1) * 256)
        # AllGather chunk order: j = 2*rank + mi -> channels 256*rank+128*mi
        wp_perm = np.concatenate(
            [wpnw[rr * 256 + mi * 128: rr * 256 + (mi + 1) * 128, rows]
             for rr in range(G) for mi in range(2)], axis=0)
        in_maps.append({
            "qT": c(q[b].T),
            "kT": c(k[b].T),
            "wq1": c(q1_w[rows].T),
            "wq2": c(q2_w[rows].T),
            "wk1": c(kv1_w[rows].T),
            "wv1": c(kv1_w[vrows].T),
            "wk2": c(kv2_w[rows].T),
            "wp": c(wp_perm),
            "pb": np.ascontiguousarray(
                proj_b[rows].reshape(2, P).T).astype(np.float32),
            "lamn": np.ascontiguousarray(
                -lam_full[g * NH:(g + 1) * NH].reshape(1, NH)
            ).astype(np.float32),
        })
    return in_maps


def kernel(**inputs):
    from concourse.bass_utils import run_bass_kernel_spmd

    nc = _get_nc()
    in_maps = _shard_inputs(inputs)
    res = run_bass_kernel_spmd(nc, in_maps, core_ids=list(range(8)))
    out = np.empty((B, NQ, DIM), np.float32)
    for r in range(8):
        b, g = r // G, r % G
        out[b, :, g * 256:(g + 1) * 256] = res.results[r]["y_out"].T
    return out
